# revision 1
# baseline (speedup 1.0000x reference)
"""Trainium2 Bass kernel for nn_LINEnew (LINE loss function).

loss = -sum(A * log_sigmoid(U1 @ U2.T)) + lmbd1 * (sum|U1| + sum|U2|)
     =  sum(A * softplus(-(U1 @ U2.T))) + lmbd1 * (sum|U1| + sum|U2|)

N=12288, D=16. Streaming A (604MB) from HBM dominates -> memory-bound.

Sharding: row-wise over 8 NeuronCores; core c owns rows [c*1536,(c+1)*1536)
of A and U1 plus a full U2^T copy. Per 128x2048 tile on each core:
  PE  : PSUM P = S - 30*A   (K=16 matmul for S = U1 U2^T, plus a -30*I
        stationary matmul streaming the A tile)
  ACT : E = exp(-P - 30) == A * exp(-S) exactly (A=0 lanes -> e^-30)
  DVE : t = (E_even + 1)*E_odd ; q = t + E_even  == (1+E0)(1+E1) - 1
  ACT : ln(q + 1) with per-partition row-sum accumulate
        == softplus(-s0) + softplus(-s1) summed pairwise (half-size pass)
L1 terms via Abs-activation accumulate; host sums [128,8] partials in f64.
"""

import sys

for _p in ("/opt/trn_rl_repo", "/root/.axon_site/_ro/trn_rl_repo"):
    if _p not in sys.path:
        sys.path.insert(0, _p)

import numpy as np

from concourse import bacc, mybir, tile
from concourse.bass_utils import run_bass_kernel_spmd

f32 = mybir.dt.float32

N = 12288
D = 16
NCORES = 8
ROWS = N // NCORES  # 1536
RT = ROWS // 128  # 12 row-tiles
ROUND = 2048  # PSUM round: 4 banks
CR = N // ROUND  # 6 col-rounds per row-tile
NMM = ROUND // 512  # 4 bank-matmuls per round
ATILE = 6144  # A DMA tile columns (3 MB per DMA)
ACR = ATILE // ROUND  # col-rounds per A tile
ACC_COLS = RT * CR  # 72
BIG = 30.0

_cache = {}


def _build_program():
    nc = bacc.Bacc("TRN2", debug=False)
    a = nc.dram_tensor("a", [ROWS, N], f32, kind="ExternalInput").ap()
    u1t = nc.dram_tensor("u1t", [D, ROWS], f32, kind="ExternalInput").ap()
    u2t = nc.dram_tensor("u2t", [D, N], f32, kind="ExternalInput").ap()
    nbi = nc.dram_tensor("nbi", [128, 128], f32, kind="ExternalInput").ap()
    res = nc.dram_tensor("res", [128, 8], f32, kind="ExternalOutput").ap()

    with tile.TileContext(nc) as tc:
        with (
            tc.tile_pool(name="const", bufs=1) as cpool,
            tc.tile_pool(name="atile", bufs=3) as apool,
            tc.tile_pool(name="es", bufs=2) as epool,
            tc.tile_pool(name="ts", bufs=2) as tpool,
            tc.tile_pool(name="qs", bufs=2) as qpool,
            tc.tile_pool(name="ps", bufs=2, space="PSUM") as pspool,
        ):
            u2t_s = cpool.tile([D, N], f32)
            nc.sync.dma_start(u2t_s, u2t)
            u1t_s = cpool.tile([D, ROWS], f32)
            nc.sync.dma_start(u1t_s, u1t)
            nbi_s = cpool.tile([128, 128], f32)
            nc.sync.dma_start(nbi_s, nbi)

            acc = cpool.tile([128, ACC_COLS], f32)
            accf = cpool.tile([128, 8], f32)
            nc.vector.memset(accf, 0.0)
            nbias = cpool.tile([128, 1], f32)
            nc.vector.memset(nbias, -BIG)

            # L1 partials: |U1 local| -> col0; |U2| (full) in chunks -> col1..6
            l1scr = cpool.tile([D, ROUND], f32)
            nc.scalar.activation(
                l1scr[:, :ROWS],
                u1t_s,
                mybir.ActivationFunctionType.Abs,
                accum_out=accf[0:D, 0:1],
            )
            for ch in range(CR):
                nc.scalar.activation(
                    l1scr,
                    u2t_s[:, ch * ROUND : (ch + 1) * ROUND],
                    mybir.ActivationFunctionType.Abs,
                    accum_out=accf[0:D, 1 + ch : 2 + ch],
                )

            for rt in range(RT):
                lhsT = u1t_s[:, rt * 128 : (rt + 1) * 128]
                for at in range(N // ATILE):
                    a_t = apool.tile([128, ATILE], f32, tag="at")
                    nc.sync.dma_start(
                        a_t,
                        a[rt * 128 : (rt + 1) * 128, at * ATILE : (at + 1) * ATILE],
                    )
                    for acr in range(ACR):
                        cr = at * ACR + acr
                        ps = pspool.tile([128, ROUND], f32)
                        for b in range(NMM):
                            nc.tensor.matmul(
                                ps[:, b * 512 : (b + 1) * 512],
                                lhsT,
                                u2t_s[:, cr * ROUND + b * 512 : cr * ROUND + (b + 1) * 512],
                                start=True,
                                stop=False,
                                skip_group_check=True,
                            )
                        for b in range(NMM):
                            nc.tensor.matmul(
                                ps[:, b * 512 : (b + 1) * 512],
                                nbi_s,
                                a_t[:, acr * ROUND + b * 512 : acr * ROUND + (b + 1) * 512],
                                start=False,
                                stop=True,
                                skip_group_check=True,
                            )
                        e_s = epool.tile([128, ROUND], f32, tag="es")
                        nc.scalar.activation(
                            e_s,
                            ps,
                            mybir.ActivationFunctionType.Exp,
                            scale=-1.0,
                            bias=nbias,
                        )
                        e3 = e_s.rearrange("p (f two) -> p f two", two=2)
                        t_s = tpool.tile([128, ROUND // 2], f32, tag="ts")
                        nc.vector.scalar_tensor_tensor(
                            out=t_s,
                            in0=e3[:, :, 0],
                            scalar=1.0,
                            in1=e3[:, :, 1],
                            op0=mybir.AluOpType.add,
                            op1=mybir.AluOpType.mult,
                        )
                        q_s = qpool.tile([128, ROUND // 2], f32, tag="qs")
                        nc.vector.tensor_tensor(
                            out=q_s,
                            in0=t_s,
                            in1=e3[:, :, 0],
                            op=mybir.AluOpType.add,
                        )
                        col = rt * CR + cr
                        nc.scalar.activation(
                            q_s,
                            q_s,
                            mybir.ActivationFunctionType.Ln,
                            bias=1.0,
                            accum_out=acc[:, col : col + 1],
                        )

            nc.vector.tensor_reduce(
                out=accf[:, 7:8],
                in_=acc[:, 0:ACC_COLS],
                axis=mybir.AxisListType.X,
                op=mybir.AluOpType.add,
            )
            nc.sync.dma_start(res, accf)
    nc.compile()
    return nc


def _run(A, U1, U2, lmbd1, trace=False):
    A = np.ascontiguousarray(np.asarray(A, dtype=np.float32))
    U1 = np.asarray(U1, dtype=np.float32)
    U2 = np.asarray(U2, dtype=np.float32)
    lmbd1 = float(np.asarray(lmbd1))

    if "nc" not in _cache:
        _cache["nc"] = _build_program()
    nc = _cache["nc"]

    u2t_full = np.ascontiguousarray(U2.T)
    nbi = (-BIG * np.eye(128)).astype(np.float32)
    in_maps = []
    for c in range(NCORES):
        r0, r1 = c * ROWS, (c + 1) * ROWS
        in_maps.append(
            {
                "a": A[r0:r1],
                "u1t": np.ascontiguousarray(U1[r0:r1].T),
                "u2t": u2t_full,
                "nbi": nbi,
            }
        )

    try:
        r = run_bass_kernel_spmd(
            nc, in_maps, core_ids=list(range(NCORES)), trace=trace
        )
    except ModuleNotFoundError:
        # NTFF profiling hook unavailable in this container; run untraced.
        r = run_bass_kernel_spmd(nc, in_maps, core_ids=list(range(NCORES)))

    main = 0.0
    l1_u1 = 0.0
    l1_u2 = 0.0
    for c in range(NCORES):
        out = r.results[c]["res"].astype(np.float64)
        main += out[:, 7].sum()
        l1_u1 += out[:, 0].sum()
        l1_u2 += out[:, 1:7].sum()
    loss = main + lmbd1 * (l1_u1 + l1_u2 / NCORES)
    return np.array(loss, dtype=np.float32), r


def kernel(A, U1, U2, lmbd1):
    return _run(A, U1, U2, lmbd1)[0]



# revision 6
# speedup vs baseline: 2.7277x; 2.7277x over previous
"""Trainium2 Bass kernel for nn_LINEnew (LINE loss function).

loss = -sum(A * log_sigmoid(U1 @ U2.T)) + lmbd1 * (sum|U1| + sum|U2|)
     =  sum(A * softplus(-(U1 @ U2.T))) + lmbd1 * (sum|U1| + sum|U2|)

N=12288, D=16. Streaming A (604MB) from HBM dominates -> memory-bound.

Sharding: row-wise over 8 NeuronCores; core c owns rows [c*1536,(c+1)*1536)
of A and U1 plus a full U2^T copy. Per 128x2048 tile on each core:
  PE  : PSUM P = S - 30*A  (f32r matmuls: K=16 for S = U1 U2^T, plus a
        -30*I stationary matmul streaming the A tile)
  ACT : E = exp(-P - 30) == A * exp(-S) (+O(e-30) dust on A=0 lanes), bf16
  DVE : p = E + 1 (4x mode); 3-level pairwise product tree
        P8 = prod over groups of 8 of (1 + E_i)   (2x tensor_tensor mults)
  ACT : ln(P8) with per-partition row-sum accumulate
        == sum softplus(-s_i) over the group's A=1 lanes (8x smaller pass)
The Ln of round r is issued after the Exp of round r+1 so the in-order ACT
queue never stalls waiting on the DVE tree.
L1 terms are O(N*D) and computed on host in f64.
"""

import sys

for _p in ("/opt/trn_rl_repo", "/root/.axon_site/_ro/trn_rl_repo"):
    if _p not in sys.path:
        sys.path.insert(0, _p)

import numpy as np

from concourse import bacc, mybir, tile
from concourse.bass_utils import run_bass_kernel_spmd
from concourse.hw_specs import get_activation_tables

f32 = mybir.dt.float32
f32r = mybir.dt.float32r
bf16 = mybir.dt.bfloat16

N = 12288
D = 16
NCORES = 8
ROWS = N // NCORES  # 1536
RT = ROWS // 128  # 12 row-tiles
ROUND = 2048  # PSUM round: 4 banks
CR = N // ROUND  # 6 col-rounds per row-tile
NMM = ROUND // 512  # 4 bank-matmuls per round
ATILE = 6144  # A DMA tile columns (3 MB per DMA)
ACR = ATILE // ROUND  # col-rounds per A tile
NROUNDS = RT * CR  # 72
BIG = 30.0

_cache = {}


def _build_program():
    nc = bacc.Bacc("TRN2", debug=False)
    a = nc.dram_tensor("a", [ROWS, N], f32r, kind="ExternalInput").ap()
    u1t = nc.dram_tensor("u1t", [D, ROWS], f32r, kind="ExternalInput").ap()
    u2t = nc.dram_tensor("u2t", [D, N], f32r, kind="ExternalInput").ap()
    nbi = nc.dram_tensor("nbi", [128, 128], f32r, kind="ExternalInput").ap()
    res = nc.dram_tensor("res", [128, 1], f32, kind="ExternalOutput").ap()

    mult = mybir.AluOpType.mult
    Exp = mybir.ActivationFunctionType.Exp
    Ln = mybir.ActivationFunctionType.Ln

    # Preload the one ACT table set that serves both Exp and Ln so the
    # act-table-load pass doesn't thrash between exp-only and ln-only sets
    # (1283 ns per reload on the ACT critical path).
    tables = list(get_activation_tables(nc.m.arch).items())
    set_id = next(
        i for i, (_, funcs) in enumerate(tables) if Exp in funcs and Ln in funcs
    )
    nc.scalar.add_instruction(
        mybir.InstLoadActFuncSet(
            name=nc.get_next_instruction_name(),
            ins=[],
            outs=[],
            act_func_set_id=set_id,
        )
    )

    with tile.TileContext(nc) as tc:
        with (
            tc.tile_pool(name="const", bufs=1) as cpool,
            tc.tile_pool(name="atile", bufs=3) as apool,
            tc.tile_pool(name="es", bufs=3) as epool,
            tc.tile_pool(name="p1", bufs=3) as ppool,
            tc.tile_pool(name="q1", bufs=3) as q1pool,
            tc.tile_pool(name="q2", bufs=3) as q2pool,
            tc.tile_pool(name="q3", bufs=3) as q3pool,
            tc.tile_pool(name="ln", bufs=2) as lpool,
            tc.tile_pool(name="ps", bufs=2, space="PSUM") as pspool,
        ):
            u2t_s = cpool.tile([D, N], f32r)
            nc.sync.dma_start(u2t_s, u2t)
            u1t_s = cpool.tile([D, ROWS], f32r)
            nc.sync.dma_start(u1t_s, u1t)
            nbi_s = cpool.tile([128, 128], f32r)
            nc.sync.dma_start(nbi_s, nbi)

            acc = cpool.tile([128, NROUNDS], f32)
            accf = cpool.tile([128, 1], f32)
            nbias = cpool.tile([128, 1], f32)
            nc.vector.memset(nbias, -BIG)

            pending = None

            def flush_pending():
                nonlocal pending
                if pending is None:
                    return
                q3p, colp = pending
                lp = lpool.tile([128, ROUND // 8], bf16, tag="l")
                nc.scalar.activation(
                    lp, q3p, Ln, accum_out=acc[:, colp : colp + 1]
                )
                pending = None

            for rt in range(RT):
                lhsT = u1t_s[:, rt * 128 : (rt + 1) * 128]
                for at in range(N // ATILE):
                    a_t = apool.tile([128, ATILE], f32r, tag="at")
                    nc.sync.dma_start(
                        a_t,
                        a[rt * 128 : (rt + 1) * 128, at * ATILE : (at + 1) * ATILE],
                    )
                    for acr in range(ACR):
                        cr = at * ACR + acr
                        col = rt * CR + cr
                        ps = pspool.tile([128, ROUND], f32, tag="ps")
                        for b in range(NMM):
                            nc.tensor.matmul(
                                ps[:, b * 512 : (b + 1) * 512],
                                lhsT,
                                u2t_s[:, cr * ROUND + b * 512 : cr * ROUND + (b + 1) * 512],
                                start=True,
                                stop=False,
                                skip_group_check=True,
                            )
                        for b in range(NMM):
                            nc.tensor.matmul(
                                ps[:, b * 512 : (b + 1) * 512],
                                nbi_s,
                                a_t[:, acr * ROUND + b * 512 : acr * ROUND + (b + 1) * 512],
                                start=False,
                                stop=True,
                                skip_group_check=True,
                            )
                        e = epool.tile([128, ROUND], bf16, tag="e")
                        nc.scalar.activation(e, ps, Exp, scale=-1.0, bias=nbias)
                        flush_pending()
                        p = ppool.tile([128, ROUND], bf16, tag="p")
                        nc.vector.tensor_scalar(
                            out=p,
                            in0=e,
                            scalar1=1.0,
                            scalar2=None,
                            op0=mybir.AluOpType.add,
                        )
                        h = ROUND // 2
                        q1 = q1pool.tile([128, h], bf16, tag="q1")
                        nc.vector.tensor_tensor(
                            out=q1, in0=p[:, :h], in1=p[:, h:], op=mult
                        )
                        q2 = q2pool.tile([128, h // 2], bf16, tag="q2")
                        nc.vector.tensor_tensor(
                            out=q2, in0=q1[:, : h // 2], in1=q1[:, h // 2 :], op=mult
                        )
                        q3 = q3pool.tile([128, h // 4], bf16, tag="q3")
                        nc.vector.tensor_tensor(
                            out=q3, in0=q2[:, : h // 4], in1=q2[:, h // 4 :], op=mult
                        )
                        pending = (q3, col)

            flush_pending()
            nc.vector.tensor_reduce(
                out=accf,
                in_=acc,
                axis=mybir.AxisListType.X,
                op=mybir.AluOpType.add,
            )
            nc.sync.dma_start(res, accf)
    nc.compile()
    return nc


def _run(A, U1, U2, lmbd1, trace=False):
    A = np.ascontiguousarray(np.asarray(A, dtype=np.float32))
    U1 = np.asarray(U1, dtype=np.float32)
    U2 = np.asarray(U2, dtype=np.float32)
    lmbd1 = float(np.asarray(lmbd1))

    if "nc" not in _cache:
        _cache["nc"] = _build_program()
    nc = _cache["nc"]

    u2t_full = np.ascontiguousarray(U2.T)
    nbi = (-BIG * np.eye(128)).astype(np.float32)
    in_maps = []
    for c in range(NCORES):
        r0, r1 = c * ROWS, (c + 1) * ROWS
        in_maps.append(
            {
                "a": A[r0:r1],
                "u1t": np.ascontiguousarray(U1[r0:r1].T),
                "u2t": u2t_full,
                "nbi": nbi,
            }
        )

    try:
        r = run_bass_kernel_spmd(
            nc, in_maps, core_ids=list(range(NCORES)), trace=trace
        )
    except ModuleNotFoundError:
        # NTFF profiling hook unavailable in this container; run untraced.
        r = run_bass_kernel_spmd(nc, in_maps, core_ids=list(range(NCORES)))

    main = 0.0
    for c in range(NCORES):
        out = r.results[c]["res"].astype(np.float64)
        main += out[:, 0].sum()
    l1 = np.abs(U1, dtype=np.float64).sum() + np.abs(U2, dtype=np.float64).sum()
    loss = main + lmbd1 * l1
    return np.array(loss, dtype=np.float32), r


def kernel(A, U1, U2, lmbd1):
    return _run(A, U1, U2, lmbd1)[0]
